# revision 24
# baseline (speedup 1.0000x reference)
"""RNNT JointNet kernel for 8 Trainium2 NeuronCores (Bass/Tile).

Math (per reference):
    enc_proj = enc @ w_enc.T          # (B,T,H)
    dec_proj = dec @ w_dec.T          # (B,U,H)
    hidden   = gelu_tanh(enc_proj[:,:,None,:] + dec_proj[:,None,:,:] + b1)
    logits   = hidden @ w2.T          # (B,T,U,V)

Sharding: 8 cores = B(4) x U-halves(2). Each core owns (b, u_half):
full T=256, U_loc=32. Weights replicated. No collectives.

The projections (671 MFLOP total) are computed on the HOST with BLAS --
only device-side work is the (B,T,U,V) logit tensor, which is 99% of
the FLOPs. This removes 1MB/core of w1 weights from the input load,
whose ~358 GB/s transfer otherwise gates the whole ramp.

Per-core dataflow:
  PE:  ~3.5us of dummy matmuls on zeros during the load phase so the
       HAM clock-gate opens (1.2 -> 2.4 GHz) before real work; then the
       big matmul with hiddenT tiles stationary:
       out[t(128), v(512)] += hidT[h,t_tile].T @ w2T[h,v].
  ACT: hiddenT = gelu(enc_pjT + bias), bias = dec_pjT[:,u] (b1 folded
       in on host) as a per-partition scalar -> fuses broadcast-add +
       gelu.
  DVE: PSUM -> SBUF casts (f32 -> bf16) per 512-col half; out tiles are
       1-bank [128,512] so 8 are in flight and the PE never waits on
       PSUM evacuation.
  DMA: ALL input loads on the sync queue in first-use order (the 16
       SDMA engines round-robin across active queues, so parallel
       queues just split the same ~358 GB/s; one ordered queue gives
       early-needed bytes full bandwidth): enc_pj, dec_pj, then w2 in 4
       contraction-chunk DMAs whose arrival order matches consumption.
       bf16 stores (host upcasts) alternate gpsimd/sync; final tiles go
       on sync/scalar so gpsimd's SWDGE drain overlaps them.
"""

import numpy as np

B, T, U, D = 4, 256, 64, 512
H, V = 512, 1024
P = 128
NH = H // P  # h chunks (contraction of the big matmul)
UL = U // 2  # U per core
N_CORES = 8

_CACHE = {}


def _build():
    import concourse.bass as bass  # noqa: F401
    import concourse.mybir as mybir
    from concourse import bacc, tile

    bf16 = mybir.dt.bfloat16
    f32 = mybir.dt.float32
    gelu = mybir.ActivationFunctionType.Gelu_apprx_tanh

    nc = bacc.Bacc(
        "TRN2",
        target_bir_lowering=False,
        debug=False,
        enable_asserts=False,
        num_devices=N_CORES,
    )

    # epj[p, i*T+t] = enc_proj[t, i*128+p]; dpj[p, i*UL+u] = dec_proj[u,
    # i*128+p] + b1[i*128+p]. w2 is i-major with full V per chunk:
    # cols [i*V + n] = w2.T[i*128+p, n].
    epj_d = nc.dram_tensor("epj", (P, NH * T), bf16, kind="ExternalInput")
    dpj_d = nc.dram_tensor("dpj", (P, NH * UL), f32, kind="ExternalInput")
    w2_d = nc.dram_tensor("w2c", (P, NH * V), bf16, kind="ExternalInput")
    out_d = nc.dram_tensor("out", (T, UL, V), bf16, kind="ExternalOutput")

    with tile.TileContext(nc) as tc:
        with (
            tc.tile_pool(name="const", bufs=1) as cpool,
            tc.tile_pool(name="hid", bufs=8) as hpool,
            tc.tile_pool(name="osb", bufs=16) as spool,
        ):
            epj_sb = cpool.tile([P, NH * T], bf16, tag="epj")
            dpj_sb = cpool.tile([P, NH * UL], f32, tag="dpj")
            w2_sb = cpool.tile([P, NH * V], bf16, tag="w2")
            dummy_sb = cpool.tile([P, 640], bf16, tag="dummy")

            nc.gpsimd.memset(dummy_sb[:], 0.0)
            nc.sync.dma_start(out=epj_sb[:], in_=epj_d.ap()[:, :])
            nc.sync.dma_start(out=dpj_sb[:], in_=dpj_d.ap()[:, :])
            # First w2 chunk split into lo/hi halves: the first matmul is
            # gated by its completion sem (data + ~2us receipt), and the
            # prologue consumes i0-lo before i0-hi.
            nc.sync.dma_start(out=w2_sb[:, 0:512], in_=w2_d.ap()[:, 0:512])
            nc.sync.dma_start(out=w2_sb[:, 512:V], in_=w2_d.ap()[:, 512:V])
            for i in range(1, NH):
                ci = slice(i * V, (i + 1) * V)
                nc.sync.dma_start(out=w2_sb[:, ci], in_=w2_d.ap()[:, ci])

            # Warmup in a scoped 1-bank PSUM scratch, freed for the out pool.
            with tc.tile_pool(name="warm_ps", bufs=1, space="PSUM") as wpool:
                warm = wpool.tile([P, 512], f32, tag="warm")
                # 11 dummies end ~contiguous with the first real matmul
                # (~12.3us), so the free-running HAM window sees no gap in
                # ANY phase -- otherwise an unlucky phase straddles the
                # dummy->real gap and ~10 real matmuls run at half clock.
                for k in range(11):
                    nc.tensor.matmul(
                        warm[:], dummy_sb[:, 0:P], dummy_sb[:, P:P + 512],
                        start=True, stop=True,
                    )

            # ---- main loop over u ----
            # 2 tags x 4 bufs = 8 one-bank tiles in flight
            with tc.tile_pool(name="out_ps", bufs=4, space="PSUM") as opool:
                def gelu_u(u):
                    hid = hpool.tile([P, NH * T], bf16, tag="hid")
                    for i in range(NH):
                        nc.scalar.activation(
                            hid[:, i * T:(i + 1) * T],
                            epj_sb[:, i * T:(i + 1) * T],
                            gelu,
                            bias=dpj_sb[:, i * UL + u: i * UL + u + 1],
                        )
                    return hid

                def evac_store(u, th, ps_lo, ps_hi):
                    ti = u * 2 + th
                    osb = spool.tile([P, V], bf16, tag="osb")
                    if ti >= 2 * UL - 2:
                        # final tiles: hi-half casts on the (now idle) ACT
                        # engine so both evacuations run in parallel with
                        # DVE, shortening the tail.
                        nc.vector.tensor_copy(osb[:, 0:512], ps_lo[:])
                        nc.scalar.activation(
                            osb[:, 512:V], ps_hi[:],
                            mybir.ActivationFunctionType.Copy)
                        if ti == 2 * UL - 1:
                            # halves stored on separate HWDGE rings
                            nc.sync.dma_start(
                                out=out_d.ap()[th * P:(th + 1) * P, u, 0:512],
                                in_=osb[:, 0:512])
                            nc.scalar.dma_start(
                                out=out_d.ap()[th * P:(th + 1) * P, u, 512:V],
                                in_=osb[:, 512:V])
                        else:
                            nc.sync.dma_start(
                                out=out_d.ap()[th * P:(th + 1) * P, u, :],
                                in_=osb[:])
                        return
                    on_sync = ti % 2 == 1 or ti >= 60
                    dma_eng = nc.sync if on_sync else nc.gpsimd
                    nc.vector.tensor_copy(osb[:, 0:512], ps_lo[:])
                    nc.vector.tensor_copy(osb[:, 512:V], ps_hi[:])
                    dma_eng.dma_start(
                        out=out_d.ap()[th * P:(th + 1) * P, u, :], in_=osb[:])

                # u=0 prologue: i-outer over both th tiles (4 parked banks)
                # so each arriving w2 chunk immediately feeds 4 matmuls
                # instead of 2 during the load phase.
                hid0 = gelu_u(0)
                pro = []
                for th in range(2):
                    p_lo = opool.tile([P, 512], f32, tag="plo")
                    p_hi = opool.tile([P, 512], f32, tag="phi")
                    pro.append((th, p_lo, p_hi))
                for i in range(NH):
                    # lo MMs for both th before hi MMs: i0-lo lands first
                    for half in range(2):
                        for th, p_lo, p_hi in pro:
                            lhsT = hid0[:, i * T + th * P: i * T + th * P + P]
                            nc.tensor.matmul(
                                (p_lo if half == 0 else p_hi)[:], lhsT,
                                w2_sb[:, i * V + half * 512:i * V + half * 512 + 512],
                                start=(i == 0), stop=(i == NH - 1))
                for th, p_lo, p_hi in pro:
                    evac_store(0, th, p_lo, p_hi)

                for u in range(1, UL):
                    hid = gelu_u(u)
                    for th in range(T // P):
                        ps_lo = opool.tile([P, 512], f32, tag="plo")  # 1 bank
                        ps_hi = opool.tile([P, 512], f32, tag="phi")  # 1 bank
                        # lo/hi interleaved per i: consumption order matches
                        # the w2 chunk arrival order during the load phase.
                        for i in range(NH):
                            lhsT = hid[:, i * T + th * P: i * T + th * P + P]
                            nc.tensor.matmul(ps_lo[:], lhsT,
                                             w2_sb[:, i * V:i * V + 512],
                                             start=(i == 0), stop=(i == NH - 1))
                            nc.tensor.matmul(ps_hi[:], lhsT,
                                             w2_sb[:, i * V + 512:(i + 1) * V],
                                             start=(i == 0), stop=(i == NH - 1))
                        evac_store(u, th, ps_lo, ps_hi)

    nc.compile()
    return nc


def _get_nc():
    if "nc" not in _CACHE:
        _CACHE["nc"] = _build()
    return _CACHE["nc"]


def _sbuf_img(mat_t):
    """[R=c*128, W] -> SBUF image [128, c*W]: img[p, c*W+w] = mat_t[c*128+p, w]."""
    r, w = mat_t.shape
    c = r // P
    return np.ascontiguousarray(
        mat_t.reshape(c, P, w).transpose(1, 0, 2).reshape(P, c * w)
    )


def _host_prep(encoder_outputs, decoder_outputs, w1, b1, w2):
    import ml_dtypes

    bf16 = ml_dtypes.bfloat16
    w_enc = w1[:, :D].astype(np.float32)   # (H, D)
    w_dec = w1[:, D:].astype(np.float32)
    w2c = _sbuf_img(w2.T.astype(bf16))     # [H,V] -> [128, NH*V]
    enc = np.asarray(encoder_outputs, dtype=np.float32)
    dec = np.asarray(decoder_outputs, dtype=np.float32)
    # host-side projections (BLAS sgemm, ~0.7 GFLOP total)
    enc_pj = np.einsum("btd,hd->bht", enc, w_enc, optimize=True)   # (B,H,T)
    dec_pj = np.einsum("bud,hd->bhu", dec, w_dec, optimize=True)   # (B,H,U)
    dec_pj += b1.astype(np.float32)[None, :, None]
    in_maps = []
    for c in range(N_CORES):
        b, uh = divmod(c, 2)
        epj = _sbuf_img(enc_pj[b].astype(bf16))             # [128, NH*T]
        dpj = _sbuf_img(np.ascontiguousarray(
            dec_pj[b, :, uh * UL:(uh + 1) * UL]))            # [128, NH*UL] f32
        in_maps.append({"epj": epj, "dpj": dpj, "w2c": w2c})
    return in_maps


def _gather(results):
    out = np.empty((B, T, U, V), dtype=np.float32)
    for c in range(N_CORES):
        b, uh = divmod(c, 2)
        out[b, :, uh * UL:(uh + 1) * UL, :] = results[c]["out"].astype(np.float32)
    return out


def kernel(encoder_outputs, decoder_outputs, w1, b1, w2):
    from concourse import bass_utils

    nc = _get_nc()
    in_maps = _host_prep(
        np.asarray(encoder_outputs), np.asarray(decoder_outputs),
        np.asarray(w1), np.asarray(b1), np.asarray(w2),
    )
    res = bass_utils.run_bass_kernel_spmd(nc, in_maps, core_ids=list(range(N_CORES)))
    return _gather(res.results)


# revision 25
# speedup vs baseline: 1.0151x; 1.0151x over previous
"""RNNT JointNet kernel for 8 Trainium2 NeuronCores (Bass/Tile).

Math (per reference):
    enc_proj = enc @ w_enc.T          # (B,T,H)
    dec_proj = dec @ w_dec.T          # (B,U,H)
    hidden   = gelu_tanh(enc_proj[:,:,None,:] + dec_proj[:,None,:,:] + b1)
    logits   = hidden @ w2.T          # (B,T,U,V)

Sharding: 8 cores = B(4) x U-halves(2). Each core owns (b, u_half):
full T=256, U_loc=32. Weights replicated. No collectives.

The projections (671 MFLOP total) are computed on the HOST with BLAS --
only device-side work is the (B,T,U,V) logit tensor, which is 99% of
the FLOPs. This removes 1MB/core of w1 weights from the input load,
whose ~358 GB/s transfer otherwise gates the whole ramp.

Per-core dataflow:
  PE:  ~3.5us of dummy matmuls on zeros during the load phase so the
       HAM clock-gate opens (1.2 -> 2.4 GHz) before real work; then the
       big matmul with hiddenT tiles stationary:
       out[t(128), v(512)] += hidT[h,t_tile].T @ w2T[h,v].
  ACT: hiddenT = gelu(enc_pjT + bias), bias = dec_pjT[:,u] (b1 folded
       in on host) as a per-partition scalar -> fuses broadcast-add +
       gelu.
  DVE: PSUM -> SBUF casts (f32 -> bf16) per 512-col half; out tiles are
       1-bank [128,512] so 8 are in flight and the PE never waits on
       PSUM evacuation.
  DMA: ALL input loads on the sync queue in first-use order (the 16
       SDMA engines round-robin across active queues, so parallel
       queues just split the same ~358 GB/s; one ordered queue gives
       early-needed bytes full bandwidth): enc_pj, dec_pj, then w2 in 4
       contraction-chunk DMAs whose arrival order matches consumption.
       bf16 stores (host upcasts) alternate gpsimd/sync; final tiles go
       on sync/scalar so gpsimd's SWDGE drain overlaps them.
"""

import numpy as np

B, T, U, D = 4, 256, 64, 512
H, V = 512, 1024
P = 128
NH = H // P  # h chunks (contraction of the big matmul)
UL = U // 2  # U per core
N_CORES = 8

_CACHE = {}


def _build():
    import concourse.bass as bass  # noqa: F401
    import concourse.mybir as mybir
    from concourse import bacc, tile

    bf16 = mybir.dt.bfloat16
    f32 = mybir.dt.float32
    gelu = mybir.ActivationFunctionType.Gelu_apprx_tanh

    nc = bacc.Bacc(
        "TRN2",
        target_bir_lowering=False,
        debug=False,
        enable_asserts=False,
        num_devices=N_CORES,
    )

    # epj[p, i*T+t] = enc_proj[t, i*128+p]; dpj[p, i*UL+u] = dec_proj[u,
    # i*128+p] + b1[i*128+p]. w2 is i-major with full V per chunk:
    # cols [i*V + n] = w2.T[i*128+p, n].
    epj_d = nc.dram_tensor("epj", (P, NH * T), bf16, kind="ExternalInput")
    dpj_d = nc.dram_tensor("dpj", (P, NH * UL), f32, kind="ExternalInput")
    w2_d = nc.dram_tensor("w2c", (P, NH * V), bf16, kind="ExternalInput")
    out_d = nc.dram_tensor("out", (T, UL, V), bf16, kind="ExternalOutput")

    with tile.TileContext(nc) as tc:
        with (
            tc.tile_pool(name="const", bufs=1) as cpool,
            tc.tile_pool(name="hid", bufs=8) as hpool,
            tc.tile_pool(name="osb", bufs=16) as spool,
        ):
            epj_sb = cpool.tile([P, NH * T], bf16, tag="epj")
            dpj_sb = cpool.tile([P, NH * UL], f32, tag="dpj")
            w2_sb = cpool.tile([P, NH * V], bf16, tag="w2")
            dummy_sb = cpool.tile([P, 640], bf16, tag="dummy")

            nc.gpsimd.memset(dummy_sb[:], 0.0)
            nc.sync.dma_start(out=epj_sb[:], in_=epj_d.ap()[:, :])
            nc.sync.dma_start(out=dpj_sb[:], in_=dpj_d.ap()[:, :])
            # First w2 chunk split into lo/hi halves: the first matmul is
            # gated by its completion sem (data + ~2us receipt), and the
            # prologue consumes i0-lo before i0-hi.
            nc.sync.dma_start(out=w2_sb[:, 0:512], in_=w2_d.ap()[:, 0:512])
            nc.sync.dma_start(out=w2_sb[:, 512:V], in_=w2_d.ap()[:, 512:V])
            for i in range(1, NH):
                ci = slice(i * V, (i + 1) * V)
                nc.sync.dma_start(out=w2_sb[:, ci], in_=w2_d.ap()[:, ci])

            # Warmup in a scoped 1-bank PSUM scratch, freed for the out pool.
            with tc.tile_pool(name="warm_ps", bufs=1, space="PSUM") as wpool:
                warm = wpool.tile([P, 512], f32, tag="warm")
                # 11 dummies end ~contiguous with the first real matmul
                # (~12.3us), so the free-running HAM window sees no gap in
                # ANY phase -- otherwise an unlucky phase straddles the
                # dummy->real gap and ~10 real matmuls run at half clock.
                for k in range(11):
                    nc.tensor.matmul(
                        warm[:], dummy_sb[:, 0:P], dummy_sb[:, P:P + 512],
                        start=True, stop=True,
                    )

            # ---- main loop over u ----
            # 2 tags x 4 bufs = 8 one-bank tiles in flight
            with tc.tile_pool(name="out_ps", bufs=4, space="PSUM") as opool:
                def gelu_u(u):
                    hid = hpool.tile([P, NH * T], bf16, tag="hid")
                    for i in range(NH):
                        nc.scalar.activation(
                            hid[:, i * T:(i + 1) * T],
                            epj_sb[:, i * T:(i + 1) * T],
                            gelu,
                            bias=dpj_sb[:, i * UL + u: i * UL + u + 1],
                        )
                    return hid

                def evac_store(u, th, ps_lo, ps_hi):
                    ti = u * 2 + th
                    osb = spool.tile([P, V], bf16, tag="osb")
                    if ti >= 2 * UL - 2:
                        # final tiles: hi-half casts on the (now idle) ACT
                        # engine so both evacuations run in parallel with
                        # DVE, shortening the tail.
                        nc.vector.tensor_copy(osb[:, 0:512], ps_lo[:])
                        nc.scalar.activation(
                            osb[:, 512:V], ps_hi[:],
                            mybir.ActivationFunctionType.Copy)
                        if ti == 2 * UL - 1:
                            # halves stored on separate HWDGE rings
                            nc.sync.dma_start(
                                out=out_d.ap()[th * P:(th + 1) * P, u, 0:512],
                                in_=osb[:, 0:512])
                            nc.scalar.dma_start(
                                out=out_d.ap()[th * P:(th + 1) * P, u, 512:V],
                                in_=osb[:, 512:V])
                        else:
                            nc.sync.dma_start(
                                out=out_d.ap()[th * P:(th + 1) * P, u, :],
                                in_=osb[:])
                        return
                    on_sync = ti % 2 == 1 or ti >= 60
                    dma_eng = nc.sync if on_sync else nc.gpsimd
                    nc.vector.tensor_copy(osb[:, 0:512], ps_lo[:])
                    nc.vector.tensor_copy(osb[:, 512:V], ps_hi[:])
                    dma_eng.dma_start(
                        out=out_d.ap()[th * P:(th + 1) * P, u, :], in_=osb[:])

                # u=0..1 prologue: i-outer over all four (u,th) tiles (8
                # parked banks) so each arriving w2 chunk immediately feeds
                # 8 matmuls during the load phase; hid(u1) is ready (~13.8us)
                # before the c1..c3 chunks land.
                hids = [gelu_u(0), gelu_u(1)]
                pro = []
                for pu in range(2):
                    for th in range(2):
                        p_lo = opool.tile([P, 512], f32, tag="plo")
                        p_hi = opool.tile([P, 512], f32, tag="phi")
                        pro.append((pu, th, p_lo, p_hi))
                for i in range(NH):
                    # lo MMs for all tiles before hi MMs: i0-lo lands first
                    for half in range(2):
                        for pu, th, p_lo, p_hi in pro:
                            lhsT = hids[pu][:, i * T + th * P: i * T + th * P + P]
                            nc.tensor.matmul(
                                (p_lo if half == 0 else p_hi)[:], lhsT,
                                w2_sb[:, i * V + half * 512:i * V + half * 512 + 512],
                                start=(i == 0), stop=(i == NH - 1))
                for pu, th, p_lo, p_hi in pro:
                    evac_store(pu, th, p_lo, p_hi)

                for u in range(2, UL):
                    hid = gelu_u(u)
                    for th in range(T // P):
                        ps_lo = opool.tile([P, 512], f32, tag="plo")  # 1 bank
                        ps_hi = opool.tile([P, 512], f32, tag="phi")  # 1 bank
                        # lo/hi interleaved per i: consumption order matches
                        # the w2 chunk arrival order during the load phase.
                        for i in range(NH):
                            lhsT = hid[:, i * T + th * P: i * T + th * P + P]
                            nc.tensor.matmul(ps_lo[:], lhsT,
                                             w2_sb[:, i * V:i * V + 512],
                                             start=(i == 0), stop=(i == NH - 1))
                            nc.tensor.matmul(ps_hi[:], lhsT,
                                             w2_sb[:, i * V + 512:(i + 1) * V],
                                             start=(i == 0), stop=(i == NH - 1))
                        evac_store(u, th, ps_lo, ps_hi)

    nc.compile()
    return nc


def _get_nc():
    if "nc" not in _CACHE:
        _CACHE["nc"] = _build()
    return _CACHE["nc"]


def _sbuf_img(mat_t):
    """[R=c*128, W] -> SBUF image [128, c*W]: img[p, c*W+w] = mat_t[c*128+p, w]."""
    r, w = mat_t.shape
    c = r // P
    return np.ascontiguousarray(
        mat_t.reshape(c, P, w).transpose(1, 0, 2).reshape(P, c * w)
    )


def _host_prep(encoder_outputs, decoder_outputs, w1, b1, w2):
    import ml_dtypes

    bf16 = ml_dtypes.bfloat16
    w_enc = w1[:, :D].astype(np.float32)   # (H, D)
    w_dec = w1[:, D:].astype(np.float32)
    w2c = _sbuf_img(w2.T.astype(bf16))     # [H,V] -> [128, NH*V]
    enc = np.asarray(encoder_outputs, dtype=np.float32)
    dec = np.asarray(decoder_outputs, dtype=np.float32)
    # host-side projections (BLAS sgemm, ~0.7 GFLOP total)
    enc_pj = np.einsum("btd,hd->bht", enc, w_enc, optimize=True)   # (B,H,T)
    dec_pj = np.einsum("bud,hd->bhu", dec, w_dec, optimize=True)   # (B,H,U)
    dec_pj += b1.astype(np.float32)[None, :, None]
    in_maps = []
    for c in range(N_CORES):
        b, uh = divmod(c, 2)
        epj = _sbuf_img(enc_pj[b].astype(bf16))             # [128, NH*T]
        dpj = _sbuf_img(np.ascontiguousarray(
            dec_pj[b, :, uh * UL:(uh + 1) * UL]))            # [128, NH*UL] f32
        in_maps.append({"epj": epj, "dpj": dpj, "w2c": w2c})
    return in_maps


def _gather(results):
    out = np.empty((B, T, U, V), dtype=np.float32)
    for c in range(N_CORES):
        b, uh = divmod(c, 2)
        out[b, :, uh * UL:(uh + 1) * UL, :] = results[c]["out"].astype(np.float32)
    return out


def kernel(encoder_outputs, decoder_outputs, w1, b1, w2):
    from concourse import bass_utils

    nc = _get_nc()
    in_maps = _host_prep(
        np.asarray(encoder_outputs), np.asarray(decoder_outputs),
        np.asarray(w1), np.asarray(b1), np.asarray(w2),
    )
    res = bass_utils.run_bass_kernel_spmd(nc, in_maps, core_ids=list(range(N_CORES)))
    return _gather(res.results)
